# revision 30
# baseline (speedup 1.0000x reference)
"""ASR CTC loss on 8 Trainium2 cores.

Strategy:
- Data-parallel: B=32 sharded 4 per core; host sums the 8 partial results.
- The log_softmax normalizer -lse[b,t] is added uniformly to every CTC state
  at step t, so it factors out of the alpha recurrence entirely: run the scan
  on RAW gathered logits, subtract sum_t lse[b,t] at the end (host side).
- Emit gather = one-hot(targets) matmul on the PE against PE-transposed logits
  tiles; the same transposed tiles feed exp+ones-matmul for the softmax
  normalizer.
- Alpha scan: parity-split states (E_j = blank state s=2j, O_j = label state
  s=2j+1), j laid on partitions (2 chunks of 128 in the free dim), batch in
  free. Cross-partition shift O_{j-1} via a PE shift-matrix matmul (+ a 1-row
  matmul for the chunk boundary). LSE2(x,y) = max(x,y) + softplus(-min(|x-y|,80))
  so the scan uses ONE activation table set (no table reloads).
"""

import numpy as np

B, T, V, L = 32, 1024, 1000, 256
TM = T - 1            # frames used (drop last): 1023
LM = L - 1            # labels used (drop first): 255
NCORES = 8
BPC = B // NCORES     # 4
NEG = -1e30
J = 256               # one-hot columns: j=0..254 labels, j=255 = blank (v=0)
NT512 = (TM + 511) // 512  # n-tiles for matmul free dim

_cache = {}
TRACE = False
LAST = None
LAST_WALL = None


def _build(tm):
    import concourse.bass as bass
    import concourse.mybir as mybir
    from concourse.tile import TileContext

    f32 = mybir.dt.float32
    Alu = mybir.AluOpType
    Act = mybir.ActivationFunctionType

    nsteps = tm - 1
    ntt = (tm + 127) // 128          # t-tiles of 128
    nvt = (V + 127) // 128           # v-chunks: 8 (last=104)
    nnt = (tm + 511) // 512          # matmul free-dim tiles

    nc = bass.Bass()
    SW = 386 + nvt + 8 + 8 + BPC * J   # statics width
    lg = nc.dram_tensor("lg", (BPC, tm + 1, V), f32, kind="ExternalInput")
    statics = nc.dram_tensor("statics", (128, SW), f32, kind="ExternalInput")
    outEO = nc.dram_tensor("outEO", (2, 128, 2, BPC), f32, kind="ExternalOutput")
    outS = nc.dram_tensor("outS", (1, BPC), f32, kind="ExternalOutput")

    with TileContext(nc) as tc:
        with (
            tc.tile_pool(name="persist", bufs=1) as P,
            tc.tile_pool(name="bigbuf", bufs=1) as BIG,
        ):
            # dependency-free dummy ACT: absorbs the one-time table load so
            # no real activation carries (table-load + data) waits
            junkA = P.tile([1, 8], f32, tag="junkA")
            nc.scalar.activation(junkA[:], junkA[:], Act.Exp)
            # single static DMA, then one DVE copy: every consumer's dep
            # becomes the DVE semaphore (instructions have ONE wait slot)
            st_sb = P.tile([128, SW], f32, tag="statics")
            nc.sync.dma_start(st_sb[:], statics[:])
            st2 = P.tile([128, SW], f32, tag="st2")
            nc.vector.tensor_copy(st2[:], st_sb[:])
            ident = st2[:, 0:128]
            shiftm = st2[:, 128:256]
            e127 = st2[:, 256:257]
            onescol = st2[:, 257:258]
            onesrow = st2[0:1, 258:386]
            o0 = 386
            vidx_sb = st2[:, o0 : o0 + nvt]
            pen_sb = st2[:, o0 + nvt : o0 + nvt + 8].rearrange(
                "p (c b) -> p c b", c=2)
            elm_sb = st2[:, o0 + nvt + 8 : o0 + nvt + 16].rearrange(
                "p (c b) -> p c b", c=2)
            tgtf_sb = st2[0:1, o0 + nvt + 16 : o0 + nvt + 16 + BPC * J].rearrange(
                "p (b j) -> p b j", b=BPC)

            # big persistent buffers
            glog = BIG.tile([128, 2, BPC, tm], f32, tag="glog")     # gathered raw logits per j
            ebb = BIG.tile([128, BPC, tm], f32, tag="ebb")          # blank logit broadcast
            lncols = BIG.tile([128, BPC, ntt], f32, tag="lncols")   # ln(sumexp) cols
            nc.vector.memset(lncols[:], 0.0)
            logT = [BIG.tile([128, tm], f32, tag=f"logT{k}", name=f"logT{k}") for k in range(nvt)]

            # ---------------- phase 1: gather + normalizer ----------------
            with (
                tc.tile_pool(name="work", bufs=2) as W,
                tc.tile_pool(name="w8", bufs=8) as W8,
                tc.tile_pool(name="wb", bufs=4) as WB,
                tc.tile_pool(name="psA", bufs=1, space="PSUM") as PSA,
                tc.tile_pool(name="psG", bufs=1, space="PSUM") as PSG,
                tc.tile_pool(name="psS", bufs=1, space="PSUM") as PSS,
            ):
                for b in range(BPC):
                    # broadcast targets row to 128 partitions
                    tbc_ps = PSA.tile([128, J], f32, tag="tps")
                    nc.tensor.matmul(tbc_ps[:], onesrow, tgtf_sb[0:1, b, :],
                                     start=True, stop=True)
                    tgt_bc = W.tile([128, J], f32, tag="tgtbc")
                    nc.vector.tensor_copy(tgt_bc[:], tbc_ps[:])

                    # transpose logits into logT[k] (v-part, t-free)
                    for tt in range(ntt):
                        t0 = tt * 128
                        tp = min(128, tm - t0)
                        nat = W8.tile([128, V], f32, tag="nat")
                        nc.sync.dma_start(nat[0:tp, :], lg[b, t0 : t0 + tp, :])
                        natc = W8.tile([128, V], f32, tag="natc")
                        nc.vector.tensor_copy(natc[0:tp, :], nat[0:tp, :])
                        for k in range(nvt):
                            v0 = k * 128
                            vp = min(128, V - v0)
                            tps = PSA.tile([128, 128], f32, tag="tps")
                            nc.tensor.transpose(tps[0:vp, 0:tp],
                                                natc[0:tp, v0 : v0 + vp],
                                                ident[0:tp, 0:tp])
                            nc.vector.tensor_copy(logT[k][0:vp, t0 : t0 + tp],
                                                  tps[0:vp, 0:tp])
                        exps = W.tile([128, V], f32, tag="exps")
                        secol = W.tile([128, 1], f32, tag="secol")
                        nc.scalar.activation(exps[0:tp, :], natc[0:tp, :], Act.Exp)
                        nc.vector.tensor_reduce(secol[0:tp, 0:1], exps[0:tp, :],
                                                mybir.AxisListType.X, Alu.add)
                        nc.scalar.activation(lncols[0:tp, b, tt : tt + 1],
                                             secol[0:tp, 0:1], Act.Ln)

                    # gather matmuls
                    gp = [[PSG.tile([128, 512], f32, tag=f"gp{m}{n}", name=f"gp{m}{n}")
                           for n in range(nnt)] for m in range(2)]
                    for k in range(nvt):
                        v0 = k * 128
                        vp = min(128, V - v0)
                        oh = W8.tile([128, J], f32, tag="oh")
                        nc.vector.tensor_tensor(
                            oh[0:vp, :], tgt_bc[0:vp, :],
                            vidx_sb[0:vp, k : k + 1].broadcast_to((vp, J)),
                            Alu.is_equal)
                        for n in range(nnt):
                            n0 = n * 512
                            npp = min(512, tm - n0)
                            for m in range(2):
                                nc.tensor.matmul(
                                    gp[m][n][:, 0:npp],
                                    oh[0:vp, m * 128 : (m + 1) * 128],
                                    logT[k][0:vp, n0 : n0 + npp],
                                    start=(k == 0), stop=(k == nvt - 1))
                    # write glog (+ label validity mask)
                    for n in range(nnt):
                        n0 = n * 512
                        npp = min(512, tm - n0)
                        for m in range(2):
                            nc.vector.tensor_tensor(
                                glog[:, m, b, n0 : n0 + npp], gp[m][n][:, 0:npp],
                                elm_sb[:, m, b : b + 1].broadcast_to((128, npp)),
                                Alu.add)
                    brow = WB.tile([1, tm], f32, tag="brow")
                    nc.sync.dma_start(brow[:], glog[127:128, 1, b, :])
                    for n in range(nnt):
                        n0 = n * 512
                        npp = min(512, tm - n0)
                        ebp = PSA.tile([128, 512], f32, tag="tps")
                        nc.tensor.matmul(ebp[:, 0:npp], onesrow,
                                         brow[0:1, n0 : n0 + npp],
                                         start=True, stop=True)
                        nc.vector.tensor_copy(ebb[:, b, n0 : n0 + npp],
                                              ebp[:, 0:npp])
                        

            # normalizer sum: S[b] = sum_t ln(sumexp[b,t])
            with tc.tile_pool(name="fin", bufs=1) as F, \
                 tc.tile_pool(name="psF", bufs=1, space="PSUM") as PSF:
                lred = F.tile([128, BPC], f32, tag="lred")
                nc.vector.tensor_reduce(lred[:], lncols[:],
                                        mybir.AxisListType.X, Alu.add)
                slp = PSF.tile([1, BPC], f32, tag="slp")
                nc.tensor.matmul(slp[:], onescol, lred[:], start=True, stop=True)
                sls = F.tile([1, BPC], f32, tag="sls")
                nc.vector.tensor_copy(sls[:], slp[:])
                nc.sync.dma_start(outS[:], sls[:])

                # ---------------- phase 2: alpha scan ----------------
                st = [F.tile([128, 2, BPC], f32, tag=f"st{i}", name=f"st{i}") for i in range(4)]
                # st[0], st[1] = E ping/pong; st[2], st[3] = O ping/pong
                nc.vector.memset(st[0][:], NEG)
                nc.vector.memset(st[2][:], NEG)
                nc.vector.tensor_copy(st[0][0:1, 0, :], ebb[0:1, :, 0])
                nc.vector.tensor_copy(st[2][0:1, 0, :], glog[0:1, 0, :, 0])

                with (
                    tc.tile_pool(name="scr", bufs=3) as S,
                    tc.tile_pool(name="psh", bufs=2, space="PSUM") as PSH,
                ):
                    for t in range(1, tm):
                        Ea, Eb = st[t % 2 ^ 1], st[t % 2]
                        Oa, Ob = st[2 + (t % 2 ^ 1)], st[2 + (t % 2)]
                        el = glog[:, :, :, t]
                        eb = ebb[:, :, t : t + 1].rearrange(
                            "p b one -> p one b").broadcast_to((128, 2, BPC))

                        osh = PSH.tile([128, 2, BPC], f32, tag="osh")
                        nc.tensor.matmul(osh[:], shiftm, Oa[:], start=True, stop=True)
                        nc.tensor.matmul(osh[0:1, 1, :], e127, Oa[:, 0, :],
                                         start=True, stop=True, skip_group_check=True)

                        t1 = S.tile([128, 2, BPC], f32, tag="t1")
                        nc.vector.tensor_tensor(t1[:], osh[:], pen_sb[:], Alu.add)
                        # maxes: m1 = max(O,E,t1) for O-path; mE = max(E,osh)
                        m1a = S.tile([128, 2, BPC], f32, tag="m1a")
                        nc.vector.tensor_tensor(m1a[:], Oa[:], Ea[:], Alu.max)
                        m1 = S.tile([128, 2, BPC], f32, tag="m1")
                        nc.vector.tensor_tensor(m1[:], m1a[:], t1[:], Alu.max)
                        mE = S.tile([128, 2, BPC], f32, tag="mE")
                        nc.vector.tensor_tensor(mE[:], Ea[:], osh[:], Alu.max)
                        ds = S.tile([128, 5, 2, BPC], f32, tag="ds")
                        nc.vector.tensor_tensor(ds[:, 0], Oa[:], m1[:], Alu.subtract)
                        nc.vector.tensor_tensor(ds[:, 1], Ea[:], m1[:], Alu.subtract)
                        nc.vector.tensor_tensor(ds[:, 2], t1[:], m1[:], Alu.subtract)
                        nc.vector.tensor_tensor(ds[:, 3], Ea[:], mE[:], Alu.subtract)
                        nc.vector.tensor_tensor(ds[:, 4], osh[:], mE[:], Alu.subtract)
                        ex = S.tile([128, 5, 2, BPC], f32, tag="ex")
                        nc.scalar.activation(ex[:], ds[:], Act.Exp)
                        lg2 = S.tile([128, 2, 2, BPC], f32, tag="lg2")
                        nc.vector.tensor_tensor(lg2[:, 0], ex[:, 0], ex[:, 1], Alu.add)
                        nc.vector.tensor_tensor(lg2[:, 0], lg2[:, 0], ex[:, 2], Alu.add)
                        nc.vector.tensor_tensor(lg2[:, 1], ex[:, 3], ex[:, 4], Alu.add)
                        ln2 = S.tile([128, 2, 2, BPC], f32, tag="ln2")
                        nc.scalar.activation(ln2[:], lg2[:], Act.Ln)
                        nO0 = S.tile([128, 2, BPC], f32, tag="nO0")
                        nc.vector.tensor_tensor(nO0[:], m1[:], ln2[:, 0], Alu.add)
                        nc.vector.tensor_tensor(Ob[:], nO0[:], el, Alu.add)
                        nE0 = S.tile([128, 2, BPC], f32, tag="nE0")
                        nc.vector.tensor_tensor(nE0[:], mE[:], ln2[:, 1], Alu.add)
                        nc.vector.tensor_tensor(Eb[:], nE0[:], eb, Alu.add)
                        # row j=0 of E: newE_0 = E_0 + eb (O_{-1} = NEG)
                        nc.vector.tensor_tensor(Eb[0:1, 0, :], Ea[0:1, 0, :],
                                                eb[0:1, 0, :], Alu.add)

                tfin = (tm - 1) % 2
                nc.sync.dma_start(outEO[0], st[tfin][:])
                nc.sync.dma_start(outEO[1], st[2 + tfin][:])
    return nc


def _sanitize_bir(bir_bytes):
    """Legalize sync waits: most TRN2 instruction structs encode ONE wait.
    Tile emits conservative wait sets; compute true vector clocks and drop
    every wait already implied by (a) the same engine's predecessor (in-order
    issue with per-op DRAIN) or (b) the remaining waits, transitively."""
    import json as _json

    bir = _json.loads(bir_bytes)
    for fn in bir.get("functions", []):
        sem_events = {}   # sem -> list of (cum_value, vc_dict)
        engine_vc = {}    # engine -> vc of its latest instruction
        sem_cum = {}      # sem -> cumulative update total so far
        for blk in fn.get("blocks", []):
            for inst in blk.get("instructions", []):
                eng = inst.get("engine", "?")
                si = inst.get("sync_info") or {}
                w = si.get("on_wait") or []
                pred = engine_vc.get(eng, {})

                def event_vc(s, v):
                    for cum, vc in sem_events.get(s, ()):
                        if cum >= v:
                            return vc
                    return None

                wvcs = []
                for ww in w:
                    s = ww.get("ant_name", "")
                    v = ww.get("wait_value", 0)
                    vc = (event_vc(s, v)
                          if ww.get("wait_mode") == "sem-ge-imm" else None)
                    wvcs.append((ww, s, v, vc))
                # iteratively drop implied waits, stalest first
                kept = list(range(len(wvcs)))
                changed = True
                while changed and len(kept) > 1:
                    changed = False
                    for i in list(kept):
                        ww, s, v, vc = wvcs[i]
                        if vc is None:
                            continue
                        cover = dict(pred)
                        for j in kept:
                            if j == i or wvcs[j][3] is None:
                                continue
                            for k2, v2 in wvcs[j][3].items():
                                if cover.get(k2, 0) < v2:
                                    cover[k2] = v2
                        if cover.get(s, 0) >= v:
                            kept.remove(i)
                            changed = True
                            break
                si["on_wait"] = [wvcs[i][0] for i in kept]
                if si.get("on_wait") or si.get("on_update"):
                    inst["sync_info"] = si
                # this instruction's vc
                myvc = dict(pred)
                for _, s, v, vc in wvcs:
                    if vc:
                        for k2, v2 in vc.items():
                            if myvc.get(k2, 0) < v2:
                                myvc[k2] = v2
                    if myvc.get(s, 0) < v:
                        myvc[s] = v
                for uu in (si.get("on_update") or []):
                    s = uu.get("ant_name", "")
                    sem_cum[s] = sem_cum.get(s, 0) + uu.get("update_value", 1)
                    myvc[s] = sem_cum[s]
                    sem_events.setdefault(s, []).append((sem_cum[s], myvc))
                engine_vc[eng] = myvc
    return _json.dumps(bir).encode()


def _patch_compilers():
    import concourse.bass_utils as bu
    import concourse.bass2jax as b2j

    if getattr(bu, "_ctc_sanitize_patched", False):
        return
    orig = bu.compile_bir_kernel

    def wrapped(bir_json, tmpdir, neff_name="file.neff"):
        return orig(_sanitize_bir(bir_json), tmpdir, neff_name)

    bu.compile_bir_kernel = wrapped
    bu._ctc_sanitize_patched = True
    if getattr(b2j, "compile_bir_kernel", None) is not None:
        b2j.compile_bir_kernel = wrapped


def _host_prep(logits, targets, target_padding_mask, tm):
    logits = np.ascontiguousarray(np.asarray(logits, dtype=np.float32))
    targets = np.asarray(targets).astype(np.int64)
    mask = np.asarray(target_padding_mask).astype(bool)
    tlen = mask.sum(axis=1).astype(np.int64) - 1          # (B,)
    tgt = targets[:, 1:]                                   # (B, 255)

    nvt = (V + 127) // 128
    vidx = (np.arange(128)[:, None] + 128 * np.arange(nvt)[None, :]).astype(np.float32)
    mats = np.zeros((128, 386), np.float32)
    mats[:, 0:128] = np.eye(128, dtype=np.float32)
    sh = np.zeros((128, 128), np.float32)
    sh[np.arange(127), np.arange(1, 128)] = 1.0            # sh[k, k+1] = 1
    mats[:, 128:256] = sh
    mats[127, 256] = 1.0
    mats[:, 257] = 1.0
    mats[:, 258:386] = 1.0

    jj = np.arange(J)  # true j; reshape(BPC,2,128) later maps j = c*128+p
    in_maps = []
    for c in range(NCORES):
        sl = slice(c * BPC, (c + 1) * BPC)
        tg = tgt[sl]                                        # (4, 255)
        tl = tlen[sl]                                       # (4,)
        tgtf = np.zeros((BPC, J), np.float32)
        tgtf[:, :LM] = tg.astype(np.float32)
        tgtf[:, LM] = 0.0                                   # blank column
        elmask = np.where(jj[None, :] < tl[:, None], 0.0, NEG).astype(np.float32)
        elmask[:, 255] = 0.0                                # keep blank row clean
        penm = np.full((BPC, J), NEG, np.float32)
        ok = (tg[:, 1:LM] != tg[:, 0 : LM - 1])             # j=1..254
        penm[:, 1:LM] = np.where(ok, 0.0, NEG)
        nvt = (V + 127) // 128
        SW = 386 + nvt + 8 + 8 + BPC * J
        statics = np.zeros((128, SW), np.float32)
        statics[:, 0:386] = mats
        o0 = 386
        statics[:, o0 : o0 + nvt] = vidx
        statics[:, o0 + nvt : o0 + nvt + 8] = (
            penm.reshape(BPC, 2, 128).transpose(2, 1, 0).reshape(128, 8))
        statics[:, o0 + nvt + 8 : o0 + nvt + 16] = (
            elmask.reshape(BPC, 2, 128).transpose(2, 1, 0).reshape(128, 8))
        statics[0, o0 + nvt + 16 :] = tgtf.reshape(-1)
        in_maps.append({"lg": logits[sl], "statics": statics})
    return in_maps, tlen


def _host_finish(results, tlen, tm):
    losses = np.zeros(B, np.float64)
    for c, res in enumerate(results):
        eo = res["outEO"].astype(np.float64)               # (2, 128, 2, 4)
        S = res["outS"].astype(np.float64)[0]              # (4,)
        aE = eo[0].transpose(1, 0, 2).reshape(256, BPC)    # [j = c*128+p, b]
        aO = eo[1].transpose(1, 0, 2).reshape(256, BPC)
        for b in range(BPC):
            gb = c * BPC + b
            tl = int(tlen[gb])
            l1 = aE[tl, b]
            l2 = aO[tl - 1, b] if tl > 0 else NEG
            m = max(l1, l2)
            lse = m + np.log(np.exp(l1 - m) + np.exp(l2 - m))
            loss = -(lse - S[b])
            if loss > 1e20:
                loss = 0.0
            losses[gb] = loss / max(tl, 1)
    return np.float32(losses.mean())


def kernel(logits, targets, target_padding_mask, tm=TM):
    from concourse.bass_utils import run_bass_kernel_spmd

    _patch_compilers()
    in_maps, tlen = _host_prep(logits, targets, target_padding_mask, tm)
    if tm not in _cache:
        _cache[tm] = _build(tm)
    nc = _cache[tm]
    import time as _time
    t0 = _time.time()
    res = run_bass_kernel_spmd(nc, in_maps, core_ids=list(range(NCORES)))
    globals()["LAST"] = res
    globals()["LAST_WALL"] = _time.time() - t0
    return _host_finish(res.results, tlen, tm)
